# revision 22
# baseline (speedup 1.0000x reference)
"""DGCNN (SGConv K=2 + conv-pool + fc) Trainium2 kernel, v4.

Math:
  A_norm = D^-1/2 (A + I) D^-1/2   (A from tril edge_w, symmetrized)
  h      = relu(A2 @ x @ lin_w + lin_b),  A2 = A_norm^2     [B, N, H]
  pooled = einsum('bnh,n->bh', h, conv_w) + conv_b; relu
  out    = pooled @ fc_w + fc_b                             [B, C]

Device strategy (data-parallel over batch, 8 cores x 512 batches):
  With c_n = conv_w[n] = s_n*|c_n| and u[b,n,h] = |c_n|*(A2 x W)[b,n,h]:
    pooled[b,h] = sum_n s_n*relu(u) = 0.5*(sum_n s_n*u + sum_n s_n*|u|)
  The linear term sum_n s_n u = (x @ (A2 c)) @ W is rank-1 in the node dim
  and is computed on host (one BLAS pass over x). The device computes only
  S[b,h] = sum_n s_n |u[b,n,h]|:

  per 8-batch group (fp16 operands, fp32 PSUM):
    MM_L x8 : z[j,(b,h)] = x_b @ lin_w      (lhsT = f-major x slice)
    ACT     : z PSUM -> SBUF fp16           (scalar engine, 570ns)
    MM_A x4 : U'[(b,h),n'] = zst_k^T @ a2p  (lhsT = zst 2-batch slice;
              a2p = A2*|c| with columns permuted sign-pos-first)
    DVE     : S via two tensor_reduce(apply_absolute_value) directly on
              PSUM over [128, 8, 128] views (pos cols add, neg cols
              negate) -- no relu crossing, no pooling matmul.
  LDWEIGHTS overlaps the previous matmul on HW (measured 64ns/pair at
  N=128), so PE ~= 33us (L) + 16.5us (A); DVE ~42us; ACT ~36.5us; DMA
  of x (fp16, 4MB chunks) ~48-52us bounds the kernel.

  Host epilogue: pooled = 0.5*(linear + S); relu(pooled + conv_b) @ fc_w
  + fc_b on [B, 64].
"""

import ml_dtypes
import numpy as np

import concourse.bacc as bacc
import concourse.bass as bass
import concourse.mybir as mybir
import concourse.tile as tile
from concourse.bass_utils import run_bass_kernel_spmd

N = 128       # nodes
F_IN = 128    # in features
H = 64        # hidden
C = 40        # classes
B = 4096      # batch
NCORES = 8
BPC = B // NCORES          # 512 batches per core
G = 8                      # batches per group (one PSUM bank of z)
NG = BPC // G              # 64 groups
# tapered x-DMA chunk sizes (in groups): big head for stream efficiency,
# small tail so the final compute groups wait on a tiny transfer
CHUNKS = [16, 16, 16, 13, 2, 1]
assert sum(CHUNKS) == NG
_starts = np.cumsum([0] + CHUNKS).tolist()
CHUNK_OF_GROUP = []
for _c, _s in enumerate(CHUNKS):
    CHUNK_OF_GROUP += [_c] * _s
PAIR = 2                   # groups per psA tile (2 banks)

F32 = mybir.dt.float32
FP16 = mybir.dt.float16

_PROG_CACHE: dict = {}
_last_in_maps: list = []
_last_ppos: int = 64

# ablation knob: 'full', 'no_p' (skip pool reduces), 'lin_only' (skip A
# stage too), 'dma_only' (skip all compute)
_VARIANT = "full"

LAGX = 1   # crossing lags L
LAGA = 2   # A-stage lags L
LAGP = 4   # pool reduces lag L (fires on odd groups)


def _build_program(has_bias: bool, ppos: int, repeat: int = 1):
    nc = bacc.Bacc(
        "TRN2", target_bir_lowering=False, debug=False, num_devices=NCORES
    )
    xP = nc.declare_dram_parameter(
        "xP", [F_IN, BPC * N], FP16, isOutput=False
    )
    a2p = nc.declare_dram_parameter("a2p", [N, N], FP16, isOutput=False)
    linw = nc.declare_dram_parameter("linw", [F_IN, H], FP16, isOutput=False)
    if has_bias:
        btile = nc.declare_dram_parameter("btile", [N, PAIR * G * H], F32,
                                          isOutput=False)
    pooled = nc.declare_dram_parameter("pooled", [N, 2 * NG * G // 2], F32,
                                       isOutput=True)  # [128, 512]

    import contextlib

    with tile.TileContext(nc) as tc:
        with (
            tc.tile_pool(name="const", bufs=1) as constp,
            tc.tile_pool(name="xin", bufs=3) as xinp,
            tc.tile_pool(name="zs", bufs=5) as zsp,
            tc.tile_pool(name="pool", bufs=2) as poolp,
            tc.tile_pool(name="ob", bufs=2) as obp,
            tc.tile_pool(name="psL", bufs=3, space="PSUM") as psL,
            tc.tile_pool(name="psA", bufs=2, space="PSUM") as psA,
        ):
            a2p_t = constp.tile([N, N], FP16)
            nc.sync.dma_start(a2p_t[:], a2p[:, :])
            linw_t = constp.tile([F_IN, H], FP16)
            nc.sync.dma_start(linw_t[:], linw[:, :])
            if has_bias:
                bt_t = constp.tile([N, PAIR * G * H], F32)
                nc.sync.dma_start(bt_t[:], btile[:, :])

            loop_cm = (
                tc.For_i(0, repeat, 1) if repeat > 1 else contextlib.nullcontext()
            )
            with loop_cm:
                X_cur: list = [None]
                zst_q: dict = {}
                ua_q: dict = {}
                pp_t = poolp.tile([N, NG * G // 2], F32, name="pp")   # [128,256]
                pn_t = poolp.tile([N, NG * G // 2], F32, name="pn")

                def stage_L(g):
                    c = CHUNK_OF_GROUP[g]
                    if g == _starts[c]:
                        ngrp = CHUNKS[c]
                        X8 = xinp.tile([F_IN, max(CHUNKS) * G * N], FP16,
                                       name="X8", tag="X")
                        nc.sync.dma_start(
                            X8[:, 0 : ngrp * G * N],
                            xP[:, _starts[c] * G * N : _starts[c + 1] * G * N],
                        )
                        X_cur[0] = X8
                    X = X_cur[0]
                    off = (g - _starts[c]) * G * N
                    zps = psL.tile([N, G * H], F32, tag="zps")
                    for b in range(G):
                        nc.tensor.matmul(
                            zps[:, b * H : (b + 1) * H],
                            lhsT=X[:, off + b * N : off + (b + 1) * N],
                            rhs=linw_t[:],
                            start=True,
                            stop=True,
                        )
                    return zps

                zps_q: dict = {}

                def stage_X(g):
                    zps = zps_q.pop(g)
                    zst = zsp.tile([N, G * H], FP16, tag="zst")
                    nc.scalar.copy(zst[:], zps[:])
                    zst_q[g] = zst

                def stage_A(g):
                    zst = zst_q.pop(g)
                    if g % PAIR == 0:
                        ua_q[g // PAIR] = psA.tile(
                            [N, PAIR * G * H], F32, name="ua", tag="ua"
                        )
                    ua = ua_q[g // PAIR]
                    base = (g % PAIR) * G * H
                    for k in range(4):
                        nc.tensor.matmul(
                            ua[:, base + k * N : base + (k + 1) * N],
                            lhsT=zst[:, k * N : (k + 1) * N],
                            rhs=a2p_t[:],
                            start=True,
                            stop=True,
                        )

                def stage_P(g):
                    # fires on pair-completing groups (odd g)
                    t = g // PAIR
                    ua = ua_q.pop(t)
                    if has_bias:
                        nc.vector.tensor_add(ua[:], ua[:], bt_t[:])
                    nch = PAIR * G * H // N  # 8 chunks of 128 per pair-tile
                    u3 = ua[:].rearrange("p (c n) -> p c n", c=nch)
                    if ppos > 0:
                        nc.vector.tensor_reduce(
                            pp_t[:, nch * t : nch * (t + 1)],
                            u3[:, :, 0:ppos],
                            axis=mybir.AxisListType.X,
                            op=mybir.AluOpType.add,
                            apply_absolute_value=True,
                        )
                    if ppos < N:
                        nc.vector.tensor_reduce(
                            pn_t[:, nch * t : nch * (t + 1)],
                            u3[:, :, ppos:N],
                            axis=mybir.AxisListType.X,
                            op=mybir.AluOpType.add,
                            apply_absolute_value=True,
                            negate=True,
                        )

                def run_full():
                    for i in range(NG + LAGP):
                        if i < NG:
                            zps_q[i] = stage_L(i)
                        if LAGX <= i < NG + LAGX:
                            stage_X(i - LAGX)
                        if LAGA <= i < NG + LAGA and _VARIANT in ("full", "no_p"):
                            stage_A(i - LAGA)
                        if i >= LAGP and (i - LAGP) % PAIR == PAIR - 1 and \
                                _VARIANT == "full":
                            stage_P(i - LAGP)
                    if _VARIANT == "full":
                        if ppos > 0:
                            nc.sync.dma_start(pooled[:, 0:256], pp_t[:])
                        if ppos < N:
                            nc.sync.dma_start(pooled[:, 256:512], pn_t[:])
                    else:
                        # bind output with a dummy write
                        ob = obp.tile([1, 512], F32, tag="obd")
                        if zst_q:
                            src = zst_q[max(zst_q)]
                            nc.vector.tensor_copy(ob[:], src[0:1, :].bitcast(F32))
                        else:
                            ua = ua_q[max(ua_q)]
                            nc.vector.tensor_copy(ob[:], ua[0:1, 0:512])
                        nc.sync.dma_start(
                            pooled[0:1, :],
                            ob[:],
                        )

                def run_dma_only():
                    for c in range(len(CHUNKS)):
                        X8 = xinp.tile([F_IN, max(CHUNKS) * G * N], FP16,
                                       name="X8d", tag="X")
                        nc.sync.dma_start(
                            X8[:, 0 : CHUNKS[c] * G * N],
                            xP[:, _starts[c] * G * N : _starts[c + 1] * G * N],
                        )
                        if c == len(CHUNKS) - 1:
                            ob = obp.tile([1, 512], F32, tag="ob")
                            nc.vector.tensor_copy(
                                ob[:], X8[0:1, 0:1024].bitcast(F32)
                            )
                            nc.sync.dma_start(pooled[0:1, :], ob[:])

                if _VARIANT == "dma_only":
                    run_dma_only()
                else:
                    run_full()
    nc.compile()
    return nc


def _get_program(has_bias: bool, ppos: int):
    key = (has_bias, ppos, _VARIANT)
    if key not in _PROG_CACHE:
        _PROG_CACHE[key] = _build_program(has_bias, ppos)
    return _PROG_CACHE[key]


def _host_adjacency(edge_w, conv_w):
    """a2p (sign-permuted A2*|c| columns), perm, v = A2 @ c, in float64."""
    ew = np.asarray(edge_w, dtype=np.float64)
    A = np.zeros((N, N), dtype=np.float64)
    xs, ys = np.tril_indices(N)
    A[xs, ys] = ew
    A = A + A.T - np.diag(np.diag(A))
    Ah = A + np.eye(N)
    deg = Ah.sum(axis=1)
    dinv = np.where(deg > 0, deg ** -0.5, 0.0)
    An = dinv[:, None] * Ah * dinv[None, :]
    A2 = An @ An
    c = np.asarray(conv_w, dtype=np.float64)
    s = np.sign(c)
    idx_pos = np.where(s > 0)[0]
    idx_neg = np.where(s <= 0)[0]
    perm = np.concatenate([idx_pos, idx_neg])
    a2c = A2 * np.abs(c)[None, :]          # a2c[j, n'] = A2[j,n']*|c_n'|
    a2p = np.ascontiguousarray(a2c[:, perm])
    v = A2 @ c
    return a2p, perm, v, len(idx_pos)


def _run(inputs: dict, trace: bool = False):
    x = np.asarray(inputs["x"], dtype=np.float32)
    edge_w = np.asarray(inputs["edge_w"], dtype=np.float32)
    lin_w = np.ascontiguousarray(np.asarray(inputs["lin_w"], dtype=np.float32))
    lin_b = np.asarray(inputs["lin_b"], dtype=np.float32)
    conv_w = np.asarray(inputs["conv_w"], dtype=np.float32)
    conv_b = np.asarray(inputs["conv_b"], dtype=np.float32)
    fc_w = np.asarray(inputs["fc_w"], dtype=np.float32)
    fc_b = np.asarray(inputs["fc_b"], dtype=np.float32)

    a2p, perm, v, ppos = _host_adjacency(edge_w, conv_w)
    has_bias = bool(np.any(lin_b != 0))
    global _last_ppos
    _last_ppos = ppos
    nc = _get_program(has_bias, ppos)

    linw_dev = lin_w.astype(np.float16)
    a2p_dev = a2p.astype(np.float16)
    in_maps = []
    for k in range(NCORES):
        xc = x[k * BPC : (k + 1) * BPC]                  # [512, j, f]
        xPk = np.ascontiguousarray(
            xc.transpose(2, 0, 1).astype(np.float16).reshape(F_IN, BPC * N)
        )  # [f, (b j)]
        m = {"xP": xPk, "a2p": a2p_dev, "linw": linw_dev}
        if has_bias:
            # btile[(b,h) partition, chunk*N + n'-idx] = lin_b[h]*|c_perm[idx]|
            cperm = np.abs(conv_w.astype(np.float64))[perm]
            bt = (
                np.tile(lin_b.astype(np.float64), 2)[:, None]
                * np.tile(cperm, PAIR * G * H // N)[None, :]
            )
            m["btile"] = np.ascontiguousarray(bt.astype(np.float32))
        in_maps.append(m)

    global _last_in_maps
    _last_in_maps = in_maps
    try:
        res = run_bass_kernel_spmd(nc, in_maps, list(range(NCORES)), trace=trace)
    except ModuleNotFoundError:
        res = run_bass_kernel_spmd(nc, in_maps, list(range(NCORES)), trace=False)

    # device S term: [128, 512] per core -> [512, 64]
    S_parts = []
    for k in range(NCORES):
        arr = res.results[k]["pooled"]               # [128, 512] f32
        Sk = np.zeros((128, 256), dtype=np.float64)
        if ppos > 0:
            Sk += arr[:, 0:256].astype(np.float64)
        if ppos < N:
            Sk += arr[:, 256:512].astype(np.float64)
        Sk = Sk.reshape(2, 64, 64, 4).transpose(2, 3, 0, 1).reshape(BPC, H)
        S_parts.append(Sk)
    S = np.concatenate(S_parts, axis=0)              # [B, H]

    # host linear term: sum_n c_n * (A2 x W + b)[b,n,h]
    r = np.tensordot(x, v.astype(np.float32), axes=([1], [0]))  # [B, F]
    linear = r.astype(np.float64) @ lin_w.astype(np.float64)    # [B, H]
    if has_bias:
        linear = linear + float(np.sum(conv_w.astype(np.float64))) * lin_b.astype(
            np.float64
        )[None, :]

    pooled = 0.5 * (linear + S) + conv_b.astype(np.float64)[0]
    p = np.maximum(pooled, 0.0).astype(np.float32)
    out = (p @ fc_w + fc_b).astype(np.float32)
    return out, res


def kernel(x, edge_w, lin_w, lin_b, conv_w, conv_b, fc_w, fc_b):
    out, _ = _run(
        {
            "x": x,
            "edge_w": edge_w,
            "lin_w": lin_w,
            "lin_b": lin_b,
            "conv_w": conv_w,
            "conv_b": conv_b,
            "fc_w": fc_w,
            "fc_b": fc_b,
        }
    )
    return out
